# revision 5
# baseline (speedup 1.0000x reference)
"""Linear-attention Trainium2 Bass kernel, v4: mixed fp8/bf16 precision,
f16 batched output, tuned queues.

Reference computation (per batch b, head h):
    qkv = x @ W^T; q,k,v -> (h, t, 64)
    k masked rows -> -inf; prepend 4 mem-kv rows
    q = softmax(q * d^-0.5, axis=feature);  k = softmax(k, axis=sequence)
    ctx = k^T v (64x64);  out = q @ ctx;  out *= mask

Sharding: 8 cores = (batch 0..3) x (head-half 0..1); each core owns one batch
and 8 heads (4 head-pairs), producing a (4096, 512) output slice. No
cross-core communication.

Precision strategy (validated host-side: ~6e-3 rel err vs 2e-2 gate):
  - Q and K projections: single-term fp8e4 DoubleRow. Their quantization
    noise washes out through the k-softmax token-average / flat q-softmax.
  - V projection: 3-term fp8 DoubleRow (x8@w8 + x8r@w8d + x8d@w8r, residuals
    scaled x16) since v errors flow straight to the output.
  - ek fp8 (averages out), vv/expq/ctxbd bf16.
  - All dequant factors folded into activation scales and the pass-B
    denominator ones-block (64.0).
"""

import numpy as np

D_MODEL = 1024
N_HEADS = 16
D_HEAD = 64
NMEM = 4
SCALE = D_HEAD ** -0.5
B = 4
L = 4096
NCORES = 8
HPC = 8            # heads per core
NPAIR = HPC // 2   # head-pairs per core
ECOLS = HPC * D_HEAD  # 512 output columns per core

WSCALE = 64.0      # host premultiplier on w_qkv before fp8 cast
RSCALE = 16.0      # residual upscale for the V projection terms
EKSCALE = 2.0      # multiplier on exp(k) via +ln2 bias (cancels in k-softmax)
CTX0DIV = 8.0      # host divides mem-kv prior by this; identity diag is 8
CTXSCALE = 64.0    # finalize multiplies normalized ctx by this
DENVAL = 64.0      # pass-B denominator ones-block value (matches CTXSCALE)

_CACHE = {}


def build_nc(seqlen=L, chunk=512):
    import concourse.tile as tile
    from concourse import bacc, mybir

    f32 = mybir.dt.float32
    f8 = mybir.dt.float8e4
    bf16 = mybir.dt.bfloat16
    AF = mybir.ActivationFunctionType
    ALU = mybir.AluOpType
    DR = mybir.MatmulPerfMode.DoubleRow

    n_chunks = seqlen // chunk
    tb_per_chunk = chunk // 128
    n_tb = seqlen // 128
    NDB2 = D_MODEL // 256  # 4 DoubleRow contraction blocks

    nc = bacc.Bacc("TRN2", target_bir_lowering=False, debug=False)

    x8 = nc.dram_tensor("x8", (n_chunks, 128, NDB2, 2, chunk), f8,
                        kind="ExternalInput").ap()
    x8r = nc.dram_tensor("x8r", (n_chunks, 128, NDB2, 2, chunk), f8,
                         kind="ExternalInput").ap()
    x8d = nc.dram_tensor("x8d", (n_chunks, 128, NDB2, 2, chunk), f8,
                         kind="ExternalInput").ap()
    wq8 = nc.dram_tensor("wq8", (128, NDB2, 2, ECOLS), f8, kind="ExternalInput").ap()
    wk8 = nc.dram_tensor("wk8", (128, NDB2, 2, ECOLS), f8, kind="ExternalInput").ap()
    wv8 = nc.dram_tensor("wv8", (128, NDB2, 2, ECOLS), f8, kind="ExternalInput").ap()
    wv8d = nc.dram_tensor("wv8d", (128, NDB2, 2, ECOLS), f8, kind="ExternalInput").ap()
    wv8r = nc.dram_tensor("wv8r", (128, NDB2, 2, ECOLS), f8, kind="ExternalInput").ap()
    ctx0b = nc.dram_tensor("ctx0b", (2, 128, 258), bf16, kind="ExternalInput").ap()
    idb = nc.dram_tensor("idb", (128, 128), bf16, kind="ExternalInput").ap()
    biasm = nc.dram_tensor("biasm", (128, n_tb), f32, kind="ExternalInput").ap()
    mf = nc.dram_tensor("mf", (128, n_tb), f32, kind="ExternalInput").ap()
    # f16 output, partition-major: [p, tb, e]; host reassembles + upcasts
    out = nc.dram_tensor("out", (128, n_tb, ECOLS), mybir.dt.float16,
                         kind="ExternalOutput").ap()

    with tile.TileContext(nc) as tc:
        with (
            tc.tile_pool(name="const", bufs=1) as cpool,
            tc.tile_pool(name="big", bufs=1) as bigpool,
            tc.tile_pool(name="small", bufs=16) as small,
            tc.tile_pool(name="xt", bufs=3) as xt_pool,
            tc.tile_pool(name="ek", bufs=2) as ek_pool,
            tc.tile_pool(name="vv", bufs=2) as vv_pool,
        ):
            # ---- first x chunk before anything else (startup latency) ----
            xts = []

            def load_chunk(ch):
                xa = xt_pool.tile([128, NDB2, 2, chunk], f8, name="xa")
                xb = xt_pool.tile([128, NDB2, 2, chunk], f8, name="xb")
                xc = xt_pool.tile([128, NDB2, 2, chunk], f8, name="xc")
                nc.sync.dma_start(out=xa, in_=x8[ch])
                nc.sync.dma_start(out=xb, in_=x8r[ch])
                nc.sync.dma_start(out=xc, in_=x8d[ch])
                return (xa, xb, xc)

            xts.append(load_chunk(0))

            # ---- constants / weights (dispatch on scalar queue) ----
            wq_sb = cpool.tile([128, NDB2, 2, ECOLS], f8, name="wq_sb", tag="wq_sb")
            wk_sb = cpool.tile([128, NDB2, 2, ECOLS], f8, name="wk_sb", tag="wk_sb")
            wv_sb = cpool.tile([128, NDB2, 2, ECOLS], f8, name="wv_sb", tag="wv_sb")
            wvd_sb = cpool.tile([128, NDB2, 2, ECOLS], f8, name="wvd_sb", tag="wvd_sb")
            wvr_sb = cpool.tile([128, NDB2, 2, ECOLS], f8, name="wvr_sb", tag="wvr_sb")
            # split constant loads across the idle vector/gpsimd queues,
            # ordered by first use (wk/wq before the v-weights)
            biasm_sb = cpool.tile([128, n_tb], f32, name="biasm_sb", tag="biasm_sb")
            mf_sb = cpool.tile([128, n_tb], f32, name="mf_sb", tag="mf_sb")
            id_sb = cpool.tile([128, 128], bf16, name="id_sb", tag="id_sb")
            ctx0_sb = cpool.tile([128, 2, 258], bf16, name="ctx0_sb", tag="ctx0_sb")
            nc.gpsimd.dma_start(out=wk_sb, in_=wk8)
            nc.gpsimd.dma_start(out=wq_sb, in_=wq8)
            nc.gpsimd.dma_start(out=wv_sb, in_=wv8)
            nc.scalar.dma_start(out=wvd_sb, in_=wv8d)
            nc.scalar.dma_start(out=wvr_sb, in_=wv8r)
            nc.scalar.dma_start(out=biasm_sb, in_=biasm)
            nc.scalar.dma_start(out=id_sb, in_=idb)
            nc.scalar.dma_start(out=ctx0_sb, in_=ctx0b.rearrange("i p c -> p i c"))
            nc.scalar.dma_start(out=mf_sb, in_=mf)

            # pass-B denominator stationary: 64.0 on each head's feature block
            ones64 = cpool.tile([128, 2], bf16, name="ones64", tag="ones64")
            nc.vector.memset(ones64, 0.0)
            nc.vector.memset(ones64[0:64, 0:1], DENVAL)
            nc.vector.memset(ones64[64:128, 1:2], DENVAL)

            # exp(q*scale) for the whole batch, resident bf16: [128, pair, t]
            expq = bigpool.tile([128, NPAIR, seqlen], bf16, name="expq", tag="expq")

            # ctx accumulators: 2 banks, each packs 2 pairs (129 cols each)
            ctx_pool_cm = tc.tile_pool(name="ctxps", bufs=1, space="PSUM")
            ctx_pool = ctx_pool_cm.__enter__()
            ctx2 = [
                ctx_pool.tile([128, 258], f32, name=f"ctx2_{i}", tag=f"ctx2_{i}")
                for i in range(2)
            ]

            def ctx_slice(g):
                return ctx2[g // 2][:, (g % 2) * 129:(g % 2) * 129 + 129]

            # ---- inject host-computed mem-kv prior (x8 identity) ----
            for i in range(2):
                nc.tensor.matmul(
                    ctx2[i], lhsT=id_sb, rhs=ctx0_sb[:, i, :],
                    start=True, stop=False, skip_group_check=True,
                )

            # ---- pass A: projections + k-softmax numerator + context ----
            with (
                tc.tile_pool(name="pq", bufs=2, space="PSUM") as pq_pool,
                tc.tile_pool(name="pk", bufs=1, space="PSUM") as pk_pool,
                tc.tile_pool(name="pv", bufs=1, space="PSUM") as pv_pool,
            ):
                for ch in range(n_chunks):
                    c0 = ch * chunk
                    if ch + 1 < n_chunks:
                        xts.append(load_chunk(ch + 1))
                    xa, xb, xc = xts[ch]

                    ek8 = vv = pq = None
                    for tbi in range(tb_per_chunk):
                        j = ch * tb_per_chunk + tbi
                        sl = tbi % 2
                        t0 = tbi * 128
                        if sl == 0:
                            ek8 = ek_pool.tile([128, 2, chunk], f8, name="ek8")
                            vv = vv_pool.tile([128, 2, NPAIR, 129], bf16, name="vv")
                            nc.gpsimd.memset(vv[:, :, :, 128:129], 1.0)
                            pq = pq_pool.tile([128, 2, chunk], f32, name="pq")

                        pk = pk_pool.tile([128, ECOLS], f32, name="pk")
                        for db2 in range(NDB2):
                            nc.tensor.matmul(
                                pk,
                                lhsT=xa[:, db2, :, t0:t0 + 128],
                                rhs=wk_sb[:, db2, :, :],
                                start=(db2 == 0), stop=(db2 == NDB2 - 1),
                                perf_mode=DR,
                            )
                        # q projection for pair g == tbi of this chunk
                        g = tbi
                        for db2 in range(NDB2):
                            nc.tensor.matmul(
                                pq[:, sl, :],
                                lhsT=wq_sb[:, db2, :, g * 128:(g + 1) * 128],
                                rhs=xa[:, db2, :, :],
                                start=(db2 == 0), stop=(db2 == NDB2 - 1),
                                perf_mode=DR,
                            )
                        # ek = 2*exp(k) (masked rows -> 0), fp8
                        nc.scalar.activation(
                            ek8[:, sl, :], pk, AF.Exp,
                            bias=biasm_sb[:, j:j + 1], scale=1.0 / WSCALE,
                        )
                        # v projection: 3 fp8 terms
                        pv = pv_pool.tile([128, ECOLS], f32, name="pv")
                        for ti, (xop, wop) in enumerate(
                            ((xa, wv_sb), (xb, wvd_sb), (xc, wvr_sb))
                        ):
                            for db2 in range(NDB2):
                                nc.tensor.matmul(
                                    pv,
                                    lhsT=xop[:, db2, :, t0:t0 + 128],
                                    rhs=wop[:, db2, :, :],
                                    start=(ti == 0 and db2 == 0),
                                    stop=(ti == 2 and db2 == NDB2 - 1),
                                    perf_mode=DR,
                                )
                        # vv = v = pv/64, bf16 (DVE: gpsimd can't read PSUM)
                        nc.vector.tensor_scalar(
                            vv[:, sl, :, 0:128],
                            pv.rearrange("p (g e) -> p g e", g=NPAIR),
                            1.0 / WSCALE, None, op0=ALU.mult,
                        )
                        if sl == 1:
                            # expq for pairs (tbi-1, tbi), bf16
                            gp = tbi - 1
                            nc.scalar.activation(
                                expq[:, gp:gp + 2, c0:c0 + chunk], pq, AF.Exp,
                                scale=SCALE / WSCALE,
                            )
                            # ctx accumulation per 128-token block (bf16)
                            for g2 in range(NPAIR):
                                for sl2 in range(2):
                                    nc.tensor.matmul(
                                        ctx_slice(g2),
                                        lhsT=ek8[:, sl2, g2 * 128:(g2 + 1) * 128],
                                        rhs=vv[:, sl2, g2, :],
                                        start=False,
                                        stop=(j == n_tb - 1 and sl2 == 1),
                                        skip_group_check=True,
                                    )

            # ---- finalize: ctxbd = 64 * ctx_num / denom, bf16 block-diag ----
            ctxbd = cpool.tile([128, NPAIR, 128], bf16, name="ctxbd", tag="ctxbd")
            nc.vector.memset(ctxbd, 0.0)
            for g in range(NPAIR):
                ps = ctx_slice(g)
                rk = small.tile([128, 1], f32, name="rk", tag="rk")
                nc.vector.reciprocal(rk, ps[:, 128:129])
                nc.vector.tensor_scalar(
                    ctxbd[0:64, g, 0:64], ps[0:64, 0:64], rk[0:64], CTXSCALE,
                    op0=ALU.mult, op1=ALU.mult,
                )
                nc.vector.tensor_scalar(
                    ctxbd[64:128, g, 64:128], ps[64:128, 64:128], rk[64:128],
                    CTXSCALE, op0=ALU.mult, op1=ALU.mult,
                )
            ctx_pool_cm.__exit__(None, None, None)

            # ---- pass B: out = (expq @ ctxbd) * recip(dn) * mask ----
            f16 = mybir.dt.float16
            with (
                tc.tile_pool(name="po", bufs=6, space="PSUM") as po_pool,
                tc.tile_pool(name="dn", bufs=2, space="PSUM") as dn_pool,
                tc.tile_pool(name="osb", bufs=4) as osb_pool,
            ):
                dn = None
                osb = None
                osb_prev = None
                pos = []
                for j in range(n_tb):
                    t0 = j * 128
                    po = po_pool.tile([128, ECOLS], f32, name="po")
                    pos.append(po)
                    if j % 2 == 0:
                        # 2-tb rr groups free po PSUM banks quickly
                        dn = dn_pool.tile([128, 4 * NPAIR], f32, name="dn")
                    if j % 4 == 0:
                        osb = osb_pool.tile([128, 4, ECOLS], f16, name="osb")
                    dof = (j % 2) * 2 * NPAIR
                    for g in range(NPAIR):
                        nc.tensor.matmul(
                            po[:, g * 128:(g + 1) * 128],
                            lhsT=expq[:, g, t0:t0 + 128],
                            rhs=ctxbd[:, g, :],
                            start=True, stop=True,
                        )
                        nc.tensor.matmul(
                            dn[:, dof + 2 * g:dof + 2 * g + 2],
                            lhsT=expq[:, g, t0:t0 + 128],
                            rhs=ones64,
                            start=True, stop=True,
                        )
                    if j % 2 == 1:
                        rr = small.tile([128, 4 * NPAIR], f32, name="rr", tag="rr")
                        nc.vector.reciprocal(rr, dn)
                        # rrm = rr * mask, so Pool needs only a tensor_tensor
                        rrm = small.tile([128, 2, 2 * NPAIR], f32,
                                         name="rrm", tag="rrm")
                        nc.gpsimd.tensor_tensor(
                            rrm,
                            rr.rearrange("p (t h) -> p t h", t=2),
                            mf_sb[:, j - 1:j + 1].broadcast_to(
                                (128, 2, 2 * NPAIR)),
                            op=ALU.mult,
                        )
                        for j2 in (j - 1, j):
                            ov = osb[:, j2 % 4, :].rearrange(
                                "p (h e) -> p h e", h=2 * NPAIR)
                            if j2 % 2 == 1 and j2 % 16 != 15:
                                # DVE reads PSUM directly
                                nc.vector.scalar_tensor_tensor(
                                    ov,
                                    pos[j2].rearrange(
                                        "p (h e) -> p h e", h=2 * NPAIR),
                                    mf_sb[:, j2:j2 + 1],
                                    rr[:, (j2 % 2) * 2 * NPAIR:
                                       (j2 % 2) * 2 * NPAIR
                                       + 2 * NPAIR].broadcast_to(
                                        (128, 2 * NPAIR, D_HEAD)),
                                    op0=ALU.mult, op1=ALU.mult,
                                )
                            else:
                                # Act bounces PSUM->SBUF; Pool (SBUF-only
                                # on HW) applies the scaling
                                posb = small.tile([128, ECOLS], f32,
                                                  name="posb", tag="posb")
                                nc.scalar.activation(
                                    posb, pos[j2], AF.Copy)
                                nc.gpsimd.tensor_tensor(
                                    ov,
                                    posb.rearrange(
                                        "p (h e) -> p h e", h=2 * NPAIR),
                                    rrm[:, j2 % 2, :].broadcast_to(
                                        (128, 2 * NPAIR, D_HEAD)),
                                    op=ALU.mult,
                                )
                    if j % 4 == 3:
                        # dispatch the PREVIOUS group's DMA now that this
                        # group's epilogues are enqueued; SP has no other
                        # pass-B work, so its dispatch waits cost nothing
                        if osb_prev is not None:
                            nc.sync.dma_start(
                                out=out[:, j - 7:j - 3, :], in_=osb_prev)
                        if j == n_tb - 1:
                            # drain the final group in 2-tb pieces
                            nc.sync.dma_start(
                                out=out[:, j - 3:j - 1, :], in_=osb[:, 0:2, :])
                            nc.scalar.dma_start(
                                out=out[:, j - 1:j + 1, :], in_=osb[:, 2:4, :])
                        else:
                            osb_prev = osb

    nc.compile()
    return nc


def _host_inputs(x, w_qkv, mem_kv, mask, seqlen=L):
    """Build the 8 per-core input maps on the host (fp8/bf16 marshalling)."""
    import ml_dtypes

    F8 = ml_dtypes.float8_e4m3
    BF16 = ml_dtypes.bfloat16
    x = np.asarray(x, dtype=np.float32)
    w_qkv = np.asarray(w_qkv, dtype=np.float32)
    mem_kv = np.asarray(mem_kv, dtype=np.float32)
    mask = np.asarray(mask)

    nb = x.shape[0]
    n_chunks = seqlen // 512
    n_tb = seqlen // 128

    def xfold(a8):
        # [1024, 4096] -> [chunk, p, db2, slot, t'] contiguous
        return np.ascontiguousarray(
            a8.reshape(4, 2, 128, n_chunks, 512).transpose(3, 2, 0, 1, 4))

    x8l, x8rl, x8dl = [], [], []
    for b in range(nb):
        xT = x[b].T  # [1024, 4096] f32
        a8 = xT.astype(F8)
        res = (xT - a8.astype(np.float32)) * RSCALE
        x8l.append(xfold(a8))
        x8rl.append(xfold(res.astype(F8)))
        x8dl.append(xfold((xT / RSCALE).astype(F8)))

    w4 = w_qkv.reshape(N_HEADS, D_HEAD, 3, D_MODEL)

    def wfold(w8):
        # [512 e, 1024 f] -> [p, db2, slot, e] contiguous
        return np.ascontiguousarray(
            w8.T.reshape(4, 2, 128, ECOLS).transpose(2, 0, 1, 3))

    wT8 = {}
    for half in (0, 1):
        h0 = half * HPC
        for ci, cn in ((0, "q"), (1, "k"), (2, "v")):
            wcol = w4[h0:h0 + HPC, :, ci, :].reshape(ECOLS, D_MODEL)
            w8 = (wcol * WSCALE).astype(F8)
            wT8[(half, cn)] = wfold(w8)
            if cn == "v":
                wT8[(half, "vd")] = wfold((wcol * WSCALE / RSCALE).astype(F8))
                res = (wcol * WSCALE - w8.astype(np.float32)) * RSCALE
                wT8[(half, "vr")] = wfold(res.astype(F8))

    idb = (np.eye(128, dtype=np.float32) * CTX0DIV).astype(BF16)

    # mem-kv prior per half: ctx0b [2, 128, 258]
    ctx0_h = {}
    for half in (0, 1):
        h0 = half * HPC
        c = np.zeros((2, 128, 258), np.float32)
        for g in range(NPAIR):
            hh = h0 + 2 * g
            mk_pair = mem_kv[0, hh:hh + 2].transpose(1, 0, 2).reshape(NMEM, 128)
            mv_pair = mem_kv[1, hh:hh + 2].transpose(1, 0, 2).reshape(NMEM, 128)
            emk2 = EKSCALE * np.exp(mk_pair)  # [4, 128]
            mve = np.concatenate([mv_pair, np.ones((NMEM, 1), np.float32)], axis=1)
            ctx0 = emk2.T @ mve  # [128, 129]
            c[g // 2, :, (g % 2) * 129:(g % 2) * 129 + 129] = ctx0 / CTX0DIV
        ctx0_h[half] = c.astype(BF16)

    ln_eks = float(np.log(EKSCALE))
    in_maps = []
    for c in range(NCORES):
        b, half = divmod(c, 2)
        mfb = mask[b].astype(np.float32)
        biasm = np.ascontiguousarray(
            (mfb * ln_eks - (1.0 - mfb) * 1e30).reshape(n_tb, 128).T)
        mfc = np.ascontiguousarray(mfb.reshape(n_tb, 128).T)
        in_maps.append({
            "x8": x8l[b],
            "x8r": x8rl[b],
            "x8d": x8dl[b],
            "wq8": wT8[(half, "q")],
            "wk8": wT8[(half, "k")],
            "wv8": wT8[(half, "v")],
            "wv8d": wT8[(half, "vd")],
            "wv8r": wT8[(half, "vr")],
            "ctx0b": ctx0_h[half],
            "idb": idb,
            "biasm": biasm,
            "mf": mfc,
        })
    return in_maps


def _get_nc():
    if "nc" not in _CACHE:
        _CACHE["nc"] = build_nc()
    return _CACHE["nc"]


def kernel(x, w_qkv, mem_kv, mask):
    from concourse.bass_utils import run_bass_kernel_spmd

    nc = _get_nc()
    in_maps = _host_inputs(x, w_qkv, mem_kv, mask)
    res = run_bass_kernel_spmd(nc, in_maps, core_ids=list(range(NCORES)))
    out = np.empty((B, L, D_MODEL), np.float32)
    for c in range(NCORES):
        b, half = divmod(c, 2)
        o16 = res.results[c]["out"]  # [128, n_tb, ECOLS] f16
        out[b, :, half * ECOLS:(half + 1) * ECOLS] = (
            o16.transpose(1, 0, 2).reshape(L, ECOLS).astype(np.float32))
    return out
